# revision 5
# baseline (speedup 1.0000x reference)
"""BCH/RS systematic encoder kernel for Trainium2 (8 NeuronCores, data parallel).

Computes out = concat([msg, (msg @ Gp) mod 2], axis=-1) for
msg [16384, 1000] f32 of 0/1 bits and Gp [1000, 256] f32 of 0/1 bits.

Design v2 (per core, 2048 rows, 8 superchunks of 2x128):
  - HWDGE plain f32 load of msg chunk straight into the f32 output-row tile
    (copy-through region), HWDGE plain f32 store of finished rows: both HBM
    transfers ride hardware-DGE rings at full rate, no SWDGE descriptor
    generation, no cast-DMA.
  - ACT casts msg f32 -> fp8e4 (0/1 exact). The fp8 tile viewed as u16 pairs
    is xbar-transposed (HALF the bytes of a bf16 transpose): each u16 holds
    msg[m, 2f] and msg[m, 2f+1], so transposed partition q carries k = 2q
    and 2q+1 interleaved along m.
  - PE DoubleRow fp8 matmuls consume exactly that pair layout: one
    instruction contracts 256 k (2 slots of 128 partitions), 4 instructions
    per 128-row chunk, accumulated f32 in PSUM (exact integer sums).
  - ACT evicts PSUM f32 -> i32, DVE ANDs with 1 (mod 2), DVE copies i32 ->
    f32 parity straight into the output-row tile.
HBM traffic/core = 8.19 MB read + 10.29 MB write (the minimum) + 2.1 MB
SBUF->SBUF transpose on the same SDMA engines.
"""

import os
import sys

import numpy as np

if os.path.isdir("/opt/trn_rl_repo") and "/opt/trn_rl_repo" not in sys.path:
    sys.path.insert(0, "/opt/trn_rl_repo")

import ml_dtypes

import concourse.bacc as bacc
import concourse.mybir as mybir
import concourse.tile as tile
from concourse.bass_utils import run_bass_kernel_spmd

BATCH = 16384
MSG = 1000
NPAR = 256
NCORES = 8
ROWS = BATCH // NCORES  # 2048
P = 128
KB = 4  # k pair-blocks of 256; padded K = 1024
KPAD = KB * 2 * P
SC = 2  # m-chunks per superchunk

# test.py pokes these for profiling
TRACE = False
LAST_RESULT = None

_CACHE = {}

F8 = mybir.dt.float8e4
U16 = mybir.dt.uint16


def build_nc(rows=ROWS):
    """Emit the Bass/Tile IR for one core handling `rows` rows."""
    mch = rows // P
    n_super = mch // SC
    nc = bacc.Bacc("TRN2", target_bir_lowering=False, debug=False)
    msg = nc.dram_tensor("msg", [rows, MSG], mybir.dt.float32, kind="ExternalInput")
    gp = nc.dram_tensor("gp", [P, KB, 2, NPAR], F8, kind="ExternalInput")
    out = nc.dram_tensor(
        "out", [rows, MSG + NPAR], mybir.dt.float32, kind="ExternalOutput"
    )

    msg3 = msg[:, :].rearrange("(s c p) k -> s c p k", c=SC, p=P)
    out3 = out[:, :].rearrange("(s c p) k -> s c p k", c=SC, p=P)

    with tile.TileContext(nc) as tc:
        with (
            tc.tile_pool(name="gpool", bufs=1) as gpool,
            # every superchunk's output-row tile is resident at once: all
            # loads dispatch upfront on the sync ring with nothing queued
            # behind them
            tc.tile_pool(name="opool", bufs=n_super) as opool,
            tc.tile_pool(name="fpool", bufs=3) as fpool,
            tc.tile_pool(name="tpool", bufs=3) as tpool,
            tc.tile_pool(name="cpool", bufs=3) as cpool,
            tc.tile_pool(name="epool", bufs=3) as epool,
            tc.tile_pool(name="ppool", bufs=4, space="PSUM") as ppool,
        ):
            # Gp resident in SBUF: gsb[q, b, j, n] = Gp_padded[256*b + 2*q + j, n]
            gsb = gpool.tile([P, KB, 2, NPAR], F8)
            nc.sync.dma_start(out=gsb[:, :, :, :], in_=gp[:, :, :, :])

            # all loads upfront; sync ring carries ONLY loads
            otiles = []
            for si in range(n_super):
                o = opool.tile([P, SC, MSG + NPAR], mybir.dt.float32, tag="o")
                nc.sync.dma_start(
                    out=o[:, :, 0:MSG],
                    in_=msg3[si, :, :, :].rearrange("c p k -> p c k"),
                )
                otiles.append(o)

            for si in range(n_super):
                o = otiles[si]
                # fp8 copy for the PE (0/1 exact); pad columns zeroed so the
                # padded k-range contributes nothing
                f8 = fpool.tile([P, SC, KPAD], F8, tag="f8")
                nc.vector.memset(f8[:, :, MSG:KPAD], 0)
                nc.scalar.copy(f8[:, :, 0:MSG], o[:, :, 0:MSG])
                # u16-pair-view xbar transpose issued from ACT right after its
                # cast (same engine stream -> no cross-engine stall), all on
                # ONE HWDGE ring (concurrent xbar transposes from two rings
                # corrupt each other):
                # t[q, c, b, m] (u16) = fp8 pair (msg[m, 256b+2q], msg[m, 256b+2q+1])
                t = tpool.tile([P, SC, KB, P], U16, tag="t")
                for c in range(SC):
                    nc.scalar.dma_start(
                        out=t[:, c, :, :],
                        in_=f8[:, c, :].bitcast(U16),
                        transpose=True,
                    )
                # plain fp8 matmuls: per pair-block b, the even/odd fp8 slot
                # of each transposed u16 is a stride-2 [128, 128] weights AP
                # contracting k = 256b + 2q + j against the matching
                # host-swizzled Gp rows
                acc = ppool.tile([P, SC * NPAR], mybir.dt.float32, tag="acc")
                for c in range(SC):
                    for b in range(KB):
                        lhsT2 = (
                            t[:, c, b, :]
                            .bitcast(F8)
                            .rearrange("q (m j) -> q j m", j=2)
                        )
                        for j in range(2):
                            nc.tensor.matmul(
                                acc[:, c * NPAR : (c + 1) * NPAR],
                                lhsT2[:, j, :],
                                gsb[:, b, j, :],
                                start=(b == 0 and j == 0),
                                stop=(b == KB - 1 and j == 1),
                            )
                # exact-integer f32 -> i32 eviction on DVE (ACT stays a pure
                # cast engine so casts never wait on the parity chain)
                ci = cpool.tile([P, SC, NPAR], mybir.dt.int32, tag="ci")
                nc.vector.tensor_copy(
                    ci[:, :, :].rearrange("p c n -> p (c n)"), acc[:, :]
                )
                # mod 2 == AND 1 (bitVec op cannot cast, keep i32)
                e = epool.tile([P, SC, NPAR], mybir.dt.int32, tag="e")
                nc.vector.tensor_scalar(
                    e[:, :, :], ci[:, :, :], 1, None, mybir.AluOpType.bitwise_and
                )
                # parity i32 -> f32 straight into the output-row tile
                nc.vector.tensor_copy(o[:, :, MSG : MSG + NPAR], e[:, :, :])
                # plain f32 store via SWDGE: its own engine stream and queue,
                # so stores never block loads/casts/transposes
                nc.gpsimd.dma_start(
                    out=out3[si, :, :, :].rearrange("c p k -> p c k"),
                    in_=o[:, :, :],
                )

    nc.compile()
    return nc


def prep_gp(Gp):
    """Pad Gp to 1024 rows and swizzle to DoubleRow [128, 4, 2, 256] fp8.

    gsw[q, b, j, n] = Gp_pad[256*b + 2*q + j, n]
    """
    gp = np.asarray(Gp, dtype=np.float32)
    gp_pad = np.zeros((KPAD, NPAR), dtype=np.float32)
    gp_pad[:MSG] = gp
    gsw = gp_pad.reshape(KB, P, 2, NPAR).transpose(1, 0, 2, 3)
    return np.ascontiguousarray(gsw).astype(ml_dtypes.float8_e4m3)


def kernel(message_bits, Gp):
    global LAST_RESULT
    msg = np.ascontiguousarray(np.asarray(message_bits, dtype=np.float32))
    assert msg.shape == (BATCH, MSG), msg.shape
    gsw = prep_gp(Gp)

    if "nc" not in _CACHE:
        _CACHE["nc"] = build_nc()
    nc = _CACHE["nc"]

    in_maps = [
        {"msg": msg[i * ROWS : (i + 1) * ROWS], "gp": gsw} for i in range(NCORES)
    ]
    res = run_bass_kernel_spmd(
        nc, in_maps, core_ids=list(range(NCORES)), trace=TRACE
    )
    LAST_RESULT = res
    return np.concatenate([r["out"] for r in res.results], axis=0)


# revision 6
# speedup vs baseline: 1.0779x; 1.0779x over previous
"""BCH/RS systematic encoder kernel for Trainium2 (8 NeuronCores, data parallel).

Computes out = concat([msg, (msg @ Gp) mod 2], axis=-1) for
msg [16384, 1000] f32 of 0/1 bits and Gp [1000, 256] f32 of 0/1 bits.

Design v5 (per core, 2048 rows, 4 supertiles of 4x128):
  - HWDGE plain f32 loads (2 MB each) straight into the f32 output-row tiles
    (copy-through region); SWDGE plain f32 stores of finished rows. Reads and
    writes share the ~358 GB/s HBM link, so the floor is (8.19+10.29)MB/358.
  - msg is cast f32 -> fp8e4 (0/1 exact), split between ACT and DVE. The fp8
    tile viewed as u16 pairs is xbar-transposed in ONE instruction per
    supertile (HALF the bytes of a bf16 transpose), dispatched from the
    otherwise-idle sync engine: transposed partition q carries k = 2q, 2q+1
    interleaved along m.
  - Plain fp8 matmuls: per 256-k pair-block, the even/odd fp8 slot of each
    u16 is a stride-2 [128,128] weights AP, contracted against host-swizzled
    Gp rows; f32 PSUM accumulation is exact.
  - ACT evicts PSUM f32 -> i32, DVE ANDs with 1 (mod 2) and copies i32 -> f32
    parity into the output-row tile.
Every engine stream carries few instructions (HWDGE dispatch costs ~1us of
issuing-engine time), so nothing serializes behind the DMA streams.
"""

import os
import sys

import numpy as np

if os.path.isdir("/opt/trn_rl_repo") and "/opt/trn_rl_repo" not in sys.path:
    sys.path.insert(0, "/opt/trn_rl_repo")

import ml_dtypes

import concourse.bacc as bacc
import concourse.mybir as mybir
import concourse.tile as tile
from concourse.bass_utils import run_bass_kernel_spmd

BATCH = 16384
MSG = 1000
NPAR = 256
NCORES = 8
ROWS = BATCH // NCORES  # 2048
P = 128
KB = 4  # k pair-blocks of 256; padded K = 1024
KPAD = KB * 2 * P
SC = 4  # m-chunks per supertile

# test.py pokes these for profiling
TRACE = False
LAST_RESULT = None

_CACHE = {}

F8 = mybir.dt.float8e4
U16 = mybir.dt.uint16


def build_nc(rows=ROWS):
    """Emit the Bass/Tile IR for one core handling `rows` rows."""
    mch = rows // P
    n_super = mch // SC
    nc = bacc.Bacc("TRN2", target_bir_lowering=False, debug=False)
    msg = nc.dram_tensor("msg", [rows, MSG], mybir.dt.float32, kind="ExternalInput")
    gp = nc.dram_tensor("gp", [P, KB, 2, NPAR], F8, kind="ExternalInput")
    out = nc.dram_tensor(
        "out", [rows, MSG + NPAR], mybir.dt.float32, kind="ExternalOutput"
    )

    msg3 = msg[:, :].rearrange("(s c p) k -> s c p k", c=SC, p=P)
    out3 = out[:, :].rearrange("(s c p) k -> s c p k", c=SC, p=P)

    ch = SC // 2  # cast split point between ACT and DVE

    with tile.TileContext(nc) as tc:
        with (
            tc.tile_pool(name="gpool", bufs=1) as gpool,
            # every supertile's output-row tile is resident at once: all
            # loads dispatch upfront on the sync ring
            tc.tile_pool(name="opool", bufs=n_super) as opool,
            tc.tile_pool(name="fpool", bufs=2) as fpool,
            tc.tile_pool(name="tpool", bufs=2) as tpool,
            tc.tile_pool(name="cpool", bufs=2) as cpool,
            tc.tile_pool(name="epool", bufs=2) as epool,
            tc.tile_pool(name="ppool", bufs=3, space="PSUM") as ppool,
        ):
            # Gp resident in SBUF: gsb[q, b, j, n] = Gp_padded[256*b + 2*q + j, n]
            gsb = gpool.tile([P, KB, 2, NPAR], F8)
            nc.sync.dma_start(out=gsb[:, :, :, :], in_=gp[:, :, :, :])

            # all loads upfront on the sync ring
            otiles = []
            for si in range(n_super):
                o = opool.tile([P, SC, MSG + NPAR], mybir.dt.float32, tag="o")
                nc.sync.dma_start(
                    out=o[:, :, 0:MSG],
                    in_=msg3[si, :, :, :].rearrange("c p k -> p c k"),
                )
                otiles.append(o)

            for si in range(n_super):
                o = otiles[si]
                # fp8 copy for the PE (0/1 exact), cast split ACT/DVE; pad
                # columns zeroed so the padded k-range contributes nothing
                f8 = fpool.tile([P, SC, KPAD], F8, tag="f8")
                nc.vector.memset(f8[:, :, MSG:KPAD], 0)
                nc.scalar.copy(f8[:, 0:ch, 0:MSG], o[:, 0:ch, 0:MSG])
                nc.vector.tensor_copy(f8[:, ch:SC, 0:MSG], o[:, ch:SC, 0:MSG])
                # ONE u16-pair-view xbar transpose per supertile, issued from
                # the sync engine (single HWDGE ring for all transposes):
                # t[q, c, b, m] (u16) = fp8 pair (msg[m, 256b+2q], msg[m, 256b+2q+1])
                t = tpool.tile([P, SC, KB, P], U16, tag="t")
                nc.sync.dma_start(
                    out=t[:, :, :, :],
                    in_=f8[:, :, :].bitcast(U16),
                    transpose=True,
                )
                # plain fp8 matmuls: per pair-block b, the even/odd fp8 slot
                # of each transposed u16 is a stride-2 [128, 128] weights AP
                # contracting k = 256b + 2q + j against the matching
                # host-swizzled Gp rows
                acc = ppool.tile([P, SC * NPAR], mybir.dt.float32, tag="acc")
                for c in range(SC):
                    for b in range(KB):
                        lhsT2 = (
                            t[:, c, b, :]
                            .bitcast(F8)
                            .rearrange("q (m j) -> q j m", j=2)
                        )
                        for j in range(2):
                            nc.tensor.matmul(
                                acc[:, c * NPAR : (c + 1) * NPAR],
                                lhsT2[:, j, :],
                                gsb[:, b, j, :],
                                start=(b == 0 and j == 0),
                                stop=(b == KB - 1 and j == 1),
                            )
                # exact-integer f32 -> i32 eviction on ACT
                ci = cpool.tile([P, SC, NPAR], mybir.dt.int32, tag="ci")
                nc.scalar.copy(ci[:, :, :].rearrange("p c n -> p (c n)"), acc[:, :])
                # mod 2 == AND 1 (bitVec op cannot cast, keep i32)
                e = epool.tile([P, SC, NPAR], mybir.dt.int32, tag="e")
                nc.vector.tensor_scalar(
                    e[:, :, :], ci[:, :, :], 1, None, mybir.AluOpType.bitwise_and
                )
                # parity i32 -> f32 straight into the output-row tile
                nc.vector.tensor_copy(o[:, :, MSG : MSG + NPAR], e[:, :, :])
                # plain f32 store via SWDGE: its own engine stream and queue
                nc.gpsimd.dma_start(
                    out=out3[si, :, :, :].rearrange("c p k -> p c k"),
                    in_=o[:, :, :],
                )

    nc.compile()
    return nc


def prep_gp(Gp):
    """Pad Gp to 1024 rows and swizzle to [128, 4, 2, 256] fp8:
    gsw[q, b, j, n] = Gp_pad[256*b + 2*q + j, n]
    """
    gp = np.asarray(Gp, dtype=np.float32)
    gp_pad = np.zeros((KPAD, NPAR), dtype=np.float32)
    gp_pad[:MSG] = gp
    gsw = gp_pad.reshape(KB, P, 2, NPAR).transpose(1, 0, 2, 3)
    return np.ascontiguousarray(gsw).astype(ml_dtypes.float8_e4m3)


def kernel(message_bits, Gp):
    global LAST_RESULT
    msg = np.ascontiguousarray(np.asarray(message_bits, dtype=np.float32))
    assert msg.shape == (BATCH, MSG), msg.shape
    gsw = prep_gp(Gp)

    if "nc" not in _CACHE:
        _CACHE["nc"] = build_nc()
    nc = _CACHE["nc"]

    in_maps = [
        {"msg": msg[i * ROWS : (i + 1) * ROWS], "gp": gsw} for i in range(NCORES)
    ]
    res = run_bass_kernel_spmd(
        nc, in_maps, core_ids=list(range(NCORES)), trace=TRACE
    )
    LAST_RESULT = res
    return np.concatenate([r["out"] for r in res.results], axis=0)


# revision 7
# speedup vs baseline: 1.1308x; 1.0491x over previous
"""BCH/RS systematic encoder kernel for Trainium2 (8 NeuronCores, data parallel).

Computes out = concat([msg, (msg @ Gp) mod 2], axis=-1) for
msg [16384, 1000] f32 of 0/1 bits and Gp [1000, 256] f32 of 0/1 bits.

Design v5 (per core, 2048 rows, 4 supertiles of 4x128):
  - HWDGE plain f32 loads (2 MB each) straight into the f32 output-row tiles
    (copy-through region); SWDGE plain f32 stores of finished rows. Reads and
    writes share the ~358 GB/s HBM link, so the floor is (8.19+10.29)MB/358.
  - msg is cast f32 -> fp8e4 (0/1 exact), split between ACT and DVE. The fp8
    tile viewed as u16 pairs is xbar-transposed in ONE instruction per
    supertile (HALF the bytes of a bf16 transpose), dispatched from the
    otherwise-idle sync engine: transposed partition q carries k = 2q, 2q+1
    interleaved along m.
  - Plain fp8 matmuls: per 256-k pair-block, the even/odd fp8 slot of each
    u16 is a stride-2 [128,128] weights AP, contracted against host-swizzled
    Gp rows; f32 PSUM accumulation is exact.
  - ACT evicts PSUM f32 -> i32, DVE ANDs with 1 (mod 2) and copies i32 -> f32
    parity into the output-row tile.
Every engine stream carries few instructions (HWDGE dispatch costs ~1us of
issuing-engine time), so nothing serializes behind the DMA streams.
"""

import os
import sys

import numpy as np

if os.path.isdir("/opt/trn_rl_repo") and "/opt/trn_rl_repo" not in sys.path:
    sys.path.insert(0, "/opt/trn_rl_repo")

import ml_dtypes

import concourse.bacc as bacc
import concourse.mybir as mybir
import concourse.tile as tile
from concourse.bass_utils import run_bass_kernel_spmd

BATCH = 16384
MSG = 1000
NPAR = 256
NCORES = 8
ROWS = BATCH // NCORES  # 2048
P = 128
KB = 4  # k pair-blocks of 256; padded K = 1024
KPAD = KB * 2 * P
SC = 4  # m-chunks per supertile

# test.py pokes these for profiling
TRACE = False
LAST_RESULT = None

_CACHE = {}

F8 = mybir.dt.float8e4
U16 = mybir.dt.uint16


def build_nc(rows=ROWS):
    """Emit the Bass/Tile IR for one core handling `rows` rows."""
    mch = rows // P
    n_super = mch // SC
    nc = bacc.Bacc("TRN2", target_bir_lowering=False, debug=False)
    msg = nc.dram_tensor("msg", [rows, MSG], mybir.dt.float32, kind="ExternalInput")
    gp = nc.dram_tensor("gp", [P, KB, 2, NPAR], F8, kind="ExternalInput")
    out = nc.dram_tensor(
        "out", [rows, MSG + NPAR], mybir.dt.float32, kind="ExternalOutput"
    )

    msg3 = msg[:, :].rearrange("(s c p) k -> s c p k", c=SC, p=P)
    out3 = out[:, :].rearrange("(s c p) k -> s c p k", c=SC, p=P)

    ch = SC // 2  # cast split point between ACT and DVE

    with tile.TileContext(nc) as tc:
        with (
            tc.tile_pool(name="gpool", bufs=1) as gpool,
            # every supertile's output-row tile is resident at once: all
            # loads dispatch upfront on the sync ring
            tc.tile_pool(name="opool", bufs=n_super) as opool,
            tc.tile_pool(name="fpool", bufs=2) as fpool,
            tc.tile_pool(name="tpool", bufs=2) as tpool,
            tc.tile_pool(name="cpool", bufs=2) as cpool,
            tc.tile_pool(name="epool", bufs=2) as epool,
            tc.tile_pool(name="ppool", bufs=3, space="PSUM") as ppool,
        ):
            # Gp resident in SBUF: gsb[q, b, j, n] = Gp_padded[256*b + 2*q + j, n]
            gsb = gpool.tile([P, KB, 2, NPAR], F8)
            nc.sync.dma_start(out=gsb[:, :, :, :], in_=gp[:, :, :, :])

            # all loads upfront on the sync ring
            otiles = []
            for si in range(n_super):
                o = opool.tile([P, SC, MSG + NPAR], mybir.dt.float32, tag="o")
                nc.sync.dma_start(
                    out=o[:, :, 0:MSG],
                    in_=msg3[si, :, :, :].rearrange("c p k -> p c k"),
                )
                otiles.append(o)

            for si in range(n_super):
                o = otiles[si]
                # fp8 copy for the PE (0/1 exact), cast split ACT/DVE; pad
                # columns zeroed so the padded k-range contributes nothing
                f8 = fpool.tile([P, SC, KPAD], F8, tag="f8")
                nc.vector.memset(f8[:, :, MSG:KPAD], 0)
                nc.scalar.copy(f8[:, 0:ch, 0:MSG], o[:, 0:ch, 0:MSG])
                nc.vector.tensor_copy(f8[:, ch:SC, 0:MSG], o[:, ch:SC, 0:MSG])
                # ONE u16-pair-view xbar transpose per supertile on the
                # scalar HWDGE ring -- its packets interleave with the load
                # packets on the sync ring instead of queuing behind them
                # (all transposes stay on this single ring):
                # t[q, c, b, m] (u16) = fp8 pair (msg[m, 256b+2q], msg[m, 256b+2q+1])
                t = tpool.tile([P, SC, KB, P], U16, tag="t")
                nc.scalar.dma_start(
                    out=t[:, :, :, :],
                    in_=f8[:, :, :].bitcast(U16),
                    transpose=True,
                )
                # plain fp8 matmuls: per pair-block b, the even/odd fp8 slot
                # of each transposed u16 is a stride-2 [128, 128] weights AP
                # contracting k = 256b + 2q + j against the matching
                # host-swizzled Gp rows
                acc = ppool.tile([P, SC * NPAR], mybir.dt.float32, tag="acc")
                for c in range(SC):
                    for b in range(KB):
                        lhsT2 = (
                            t[:, c, b, :]
                            .bitcast(F8)
                            .rearrange("q (m j) -> q j m", j=2)
                        )
                        for j in range(2):
                            nc.tensor.matmul(
                                acc[:, c * NPAR : (c + 1) * NPAR],
                                lhsT2[:, j, :],
                                gsb[:, b, j, :],
                                start=(b == 0 and j == 0),
                                stop=(b == KB - 1 and j == 1),
                            )
                # exact-integer f32 -> i32 eviction on ACT
                ci = cpool.tile([P, SC, NPAR], mybir.dt.int32, tag="ci")
                nc.scalar.copy(ci[:, :, :].rearrange("p c n -> p (c n)"), acc[:, :])
                # mod 2 == AND 1 (bitVec op cannot cast, keep i32)
                e = epool.tile([P, SC, NPAR], mybir.dt.int32, tag="e")
                nc.vector.tensor_scalar(
                    e[:, :, :], ci[:, :, :], 1, None, mybir.AluOpType.bitwise_and
                )
                # parity i32 -> f32 straight into the output-row tile
                nc.vector.tensor_copy(o[:, :, MSG : MSG + NPAR], e[:, :, :])
                # plain f32 store via SWDGE: its own engine stream and queue
                nc.gpsimd.dma_start(
                    out=out3[si, :, :, :].rearrange("c p k -> p c k"),
                    in_=o[:, :, :],
                )

    nc.compile()
    return nc


def prep_gp(Gp):
    """Pad Gp to 1024 rows and swizzle to [128, 4, 2, 256] fp8:
    gsw[q, b, j, n] = Gp_pad[256*b + 2*q + j, n]
    """
    gp = np.asarray(Gp, dtype=np.float32)
    gp_pad = np.zeros((KPAD, NPAR), dtype=np.float32)
    gp_pad[:MSG] = gp
    gsw = gp_pad.reshape(KB, P, 2, NPAR).transpose(1, 0, 2, 3)
    return np.ascontiguousarray(gsw).astype(ml_dtypes.float8_e4m3)


def kernel(message_bits, Gp):
    global LAST_RESULT
    msg = np.ascontiguousarray(np.asarray(message_bits, dtype=np.float32))
    assert msg.shape == (BATCH, MSG), msg.shape
    gsw = prep_gp(Gp)

    if "nc" not in _CACHE:
        _CACHE["nc"] = build_nc()
    nc = _CACHE["nc"]

    in_maps = [
        {"msg": msg[i * ROWS : (i + 1) * ROWS], "gp": gsw} for i in range(NCORES)
    ]
    res = run_bass_kernel_spmd(
        nc, in_maps, core_ids=list(range(NCORES)), trace=TRACE
    )
    LAST_RESULT = res
    return np.concatenate([r["out"] for r in res.results], axis=0)


# revision 11
# speedup vs baseline: 1.6036x; 1.4181x over previous
"""BCH/RS systematic encoder kernel for Trainium2 (8 NeuronCores, data parallel).

Computes out = concat([msg, (msg @ Gp) mod 2], axis=-1) for
msg [16384, 1000] f32 of 0/1 bits and Gp [1000, 256] f32 of 0/1 bits.

Design v7 (per core, 2048 rows, 4 supertiles of 4x128):
  - HWDGE plain f32 loads (2 MB each) straight into the f32 output-row tiles
    (copy-through region); SWDGE plain f32 stores of finished rows. Reads and
    writes share the ~358 GB/s HBM link: (8.19+10.29)MB/358 ~= 52 us is the
    floor, and with NO other DMA traffic the two streams fully interleave.
    (Tile serializes any xbar-transpose DMA against ALL concurrent DMAs --
    v5/v6 traces showed loads/stores blocked on transpose completion
    semaphores -- so all transposing happens on the PE instead.)
  - msg is cast f32 -> fp8e4 (0/1 exact), split between ACT and DVE.
  - The fp8 tile viewed as u16 PAIRS is transposed on the otherwise-idle PE
    (nc.tensor.transpose against a host-loaded identity), 16 blocks per
    supertile into PSUM, then evicted to SBUF by ACT: transposed partition q
    carries k = 2q, 2q+1 interleaved along m. u16 (not bf16) so no FTZ/NaN
    canonicalization can touch the fp8 pair bits.
  - Plain fp8 matmuls: per 256-k pair-block, the even/odd fp8 slot of each
    u16 is a stride-2 [128,128] weights AP, contracted against host-swizzled
    Gp rows; f32 PSUM accumulation is exact.
  - DVE evicts parity PSUM f32 -> i32, ANDs with 1 (mod 2), copies i32 -> f32
    into the output-row tile.
"""

import os
import sys

import numpy as np

if os.path.isdir("/opt/trn_rl_repo") and "/opt/trn_rl_repo" not in sys.path:
    sys.path.insert(0, "/opt/trn_rl_repo")

import ml_dtypes

import concourse.bacc as bacc
import concourse.mybir as mybir
import concourse.tile as tile
from concourse.bass_utils import run_bass_kernel_spmd

BATCH = 16384
MSG = 1000
NPAR = 256
NCORES = 8
ROWS = BATCH // NCORES  # 2048
P = 128
KB = 4  # k pair-blocks of 256; padded K = 1024
KPAD = KB * 2 * P
SC = 4  # m-chunks per supertile

# test.py pokes these for profiling
TRACE = False
LAST_RESULT = None

_CACHE = {}

F8 = mybir.dt.float8e4
U16 = mybir.dt.uint16


def build_nc(rows=ROWS):
    """Emit the Bass/Tile IR for one core handling `rows` rows."""
    mch = rows // P
    n_super = mch // SC
    nc = bacc.Bacc("TRN2", target_bir_lowering=False, debug=False)
    msg = nc.dram_tensor("msg", [rows, MSG], mybir.dt.float32, kind="ExternalInput")
    gp = nc.dram_tensor("gp", [P, KB, 2, NPAR], F8, kind="ExternalInput")
    ident = nc.dram_tensor("ident", [P, P], F8, kind="ExternalInput")
    out = nc.dram_tensor(
        "out", [rows, MSG + NPAR], mybir.dt.float32, kind="ExternalOutput"
    )

    msg3 = msg[:, :].rearrange("(s c p) k -> s c p k", c=SC, p=P)
    out3 = out[:, :].rearrange("(s c p) k -> s c p k", c=SC, p=P)

    ch = SC // 2  # cast split point between ACT and DVE

    with tile.TileContext(nc) as tc:
        with (
            tc.tile_pool(name="gpool", bufs=1) as gpool,
            # every supertile's output-row tile is resident at once: all
            # loads dispatch upfront on the sync ring
            tc.tile_pool(name="opool", bufs=n_super) as opool,
            tc.tile_pool(name="fpool", bufs=2) as fpool,
            tc.tile_pool(name="tpool", bufs=2) as tpool,
            tc.tile_pool(name="cpool", bufs=2) as cpool,
            tc.tile_pool(name="epool", bufs=2) as epool,
            tc.tile_pool(name="tppool", bufs=2, space="PSUM") as tppool,
            tc.tile_pool(name="ppool", bufs=2, space="PSUM") as ppool,
        ):
            # Gp resident in SBUF: gsb[q, b, j, n] = Gp_padded[256*b + 2*q + j, n]
            gsb = gpool.tile([P, KB, 2, NPAR], F8)
            nc.sync.dma_start(out=gsb[:, :, :, :], in_=gp[:, :, :, :])
            idsb = gpool.tile([P, P], F8)
            nc.sync.dma_start(out=idsb[:, :], in_=ident[:, :])

            # all loads upfront on the sync ring (nothing else rides it)
            otiles = []
            for si in range(n_super):
                o = opool.tile([P, SC, MSG + NPAR], mybir.dt.float32, tag="o")
                nc.sync.dma_start(
                    out=o[:, :, 0:MSG],
                    in_=msg3[si, :, :, :].rearrange("c p k -> p c k"),
                )
                otiles.append(o)

            f8s, tps, ts, accs = {}, {}, {}, {}

            def emit_prep(si):
                # fp8 copy for the PE (0/1 exact), cast split ACT/DVE; pad
                # columns zeroed so the padded k-range contributes nothing
                f8 = fpool.tile([P, SC, KPAD], F8, tag="f8")
                nc.vector.memset(f8[:, :, MSG:KPAD], 0)
                nc.scalar.copy(f8[:, 0:ch, 0:MSG], otiles[si][:, 0:ch, 0:MSG])
                nc.vector.tensor_copy(
                    f8[:, ch:SC, 0:MSG], otiles[si][:, ch:SC, 0:MSG]
                )
                f8s[si] = f8

            def emit_transpose(si, h):
                # PE transpose of plain fp8 blocks -> PSUM (fp8 0/1 bytes are
                # always normal numbers -- no FTZ hazard). The fp8 transpose
                # datapath writes one value per 16-bit lane, so the output AP
                # has element step 2: tp[q, s, 2m] = msg[m, k-block s]
                f8 = f8s[si]
                tp = tppool.tile([P, 2 * 8, 2 * P], F8, tag="tp")
                for i, c in enumerate(range(2 * h, 2 * h + 2)):
                    for blk in range(8):
                        nc.tensor.transpose(
                            tp[:, i * 8 + blk, :].rearrange(
                                "q (m two) -> q m two", two=2
                            )[:, :, 0],
                            f8[:, c, 128 * blk : 128 * (blk + 1)],
                            idsb[:, :],
                        )
                tps[(si, h)] = tp

            def emit_evict(si, h):
                # transposed blocks PSUM -> SBUF on ACT (gather even bytes)
                if h == 0:
                    t = tpool.tile([P, SC * 8, P], F8, tag="t")
                    ts[si] = t
                t = ts[si]
                nc.scalar.copy(
                    t[:, 16 * h : 16 * (h + 1), :],
                    tps.pop((si, h))[:, :, :].rearrange(
                        "q s (m two) -> q s m two", two=2
                    )[:, :, :, 0],
                )

            def emit_matmul(si):
                # DoubleRow fp8 matmuls: two adjacent true-transposed blocks
                # form the [128, 2, 128] block-layout weights AP (the layout
                # the dual-fp8 LDW ISA accepts), contracting k = 256g + 128i + q
                # against the matching host-swizzled Gp rows
                t = ts[si]
                acc = ppool.tile([P, SC * NPAR], mybir.dt.float32, tag="acc")
                for c in range(SC):
                    for g in range(KB):
                        nc.tensor.matmul(
                            acc[:, c * NPAR : (c + 1) * NPAR],
                            t[:, c * 8 + 2 * g : c * 8 + 2 * g + 2, :],
                            gsb[:, g, :, :],
                            start=(g == 0),
                            stop=(g == KB - 1),
                            perf_mode=mybir.MatmulPerfMode.DoubleRow,
                        )
                accs[si] = acc

            def emit_parity_store(si):
                o = otiles[si]
                # exact-integer f32 -> i32 eviction, mod 2 == AND 1, parity
                # i32 -> f32 into the output-row tile: all DVE
                ci = cpool.tile([P, SC, NPAR], mybir.dt.int32, tag="ci")
                nc.vector.tensor_copy(
                    ci[:, :, :].rearrange("p c n -> p (c n)"), accs.pop(si)[:, :]
                )
                e = epool.tile([P, SC, NPAR], mybir.dt.int32, tag="e")
                nc.vector.tensor_scalar(
                    e[:, :, :], ci[:, :, :], 1, None, mybir.AluOpType.bitwise_and
                )
                nc.vector.tensor_copy(o[:, :, MSG : MSG + NPAR], e[:, :, :])
                # plain f32 store via SWDGE: its own engine stream and queue
                nc.gpsimd.dma_start(
                    out=out3[si, :, :, :].rearrange("c p k -> p c k"),
                    in_=o[:, :, :],
                )

            # software-pipelined emission: transpose(si+1) is emitted before
            # matmul(si) so the PE never sits behind a cross-engine evict
            emit_prep(0)
            emit_transpose(0, 0)
            emit_transpose(0, 1)
            for si in range(n_super):
                if si + 1 < n_super:
                    emit_prep(si + 1)
                emit_evict(si, 0)
                emit_evict(si, 1)
                if si + 1 < n_super:
                    emit_transpose(si + 1, 0)
                    emit_transpose(si + 1, 1)
                emit_matmul(si)
                emit_parity_store(si)

    nc.compile()
    return nc


def prep_gp(Gp):
    """Pad Gp to 1024 rows and swizzle to [128, 4, 2, 256] fp8:
    gsw[q, g, i, n] = Gp_pad[256*g + 128*i + q, n]
    """
    gp = np.asarray(Gp, dtype=np.float32)
    gp_pad = np.zeros((KPAD, NPAR), dtype=np.float32)
    gp_pad[:MSG] = gp
    gsw = gp_pad.reshape(KB, 2, P, NPAR).transpose(2, 0, 1, 3)
    return np.ascontiguousarray(gsw).astype(ml_dtypes.float8_e4m3)


def kernel(message_bits, Gp):
    global LAST_RESULT
    msg = np.ascontiguousarray(np.asarray(message_bits, dtype=np.float32))
    assert msg.shape == (BATCH, MSG), msg.shape
    gsw = prep_gp(Gp)
    ident = np.eye(P, dtype=np.float32).astype(ml_dtypes.float8_e4m3)

    if "nc" not in _CACHE:
        _CACHE["nc"] = build_nc()
    nc = _CACHE["nc"]

    in_maps = [
        {"msg": msg[i * ROWS : (i + 1) * ROWS], "gp": gsw, "ident": ident}
        for i in range(NCORES)
    ]
    res = run_bass_kernel_spmd(
        nc, in_maps, core_ids=list(range(NCORES)), trace=TRACE
    )
    LAST_RESULT = res
    return np.concatenate([r["out"] for r in res.results], axis=0)


# revision 12
# speedup vs baseline: 1.6406x; 1.0231x over previous
"""BCH/RS systematic encoder kernel for Trainium2 (8 NeuronCores, data parallel).

Computes out = concat([msg, (msg @ Gp) mod 2], axis=-1) for
msg [16384, 1000] f32 of 0/1 bits and Gp [1000, 256] f32 of 0/1 bits.

Design v7 (per core, 2048 rows, 4 supertiles of 4x128):
  - HWDGE plain f32 loads (2 MB each) straight into the f32 output-row tiles
    (copy-through region); SWDGE plain f32 stores of finished rows. Reads and
    writes share the ~358 GB/s HBM link: (8.19+10.29)MB/358 ~= 52 us is the
    floor, and with NO other DMA traffic the two streams fully interleave.
    (Tile serializes any xbar-transpose DMA against ALL concurrent DMAs --
    v5/v6 traces showed loads/stores blocked on transpose completion
    semaphores -- so all transposing happens on the PE instead.)
  - msg is cast f32 -> fp8e4 (0/1 exact), split between ACT and DVE.
  - The fp8 tile viewed as u16 PAIRS is transposed on the otherwise-idle PE
    (nc.tensor.transpose against a host-loaded identity), 16 blocks per
    supertile into PSUM, then evicted to SBUF by ACT: transposed partition q
    carries k = 2q, 2q+1 interleaved along m. u16 (not bf16) so no FTZ/NaN
    canonicalization can touch the fp8 pair bits.
  - Plain fp8 matmuls: per 256-k pair-block, the even/odd fp8 slot of each
    u16 is a stride-2 [128,128] weights AP, contracted against host-swizzled
    Gp rows; f32 PSUM accumulation is exact.
  - DVE evicts parity PSUM f32 -> i32, ANDs with 1 (mod 2), copies i32 -> f32
    into the output-row tile.
"""

import os
import sys

import numpy as np

if os.path.isdir("/opt/trn_rl_repo") and "/opt/trn_rl_repo" not in sys.path:
    sys.path.insert(0, "/opt/trn_rl_repo")

import ml_dtypes

import concourse.bacc as bacc
import concourse.mybir as mybir
import concourse.tile as tile
from concourse.bass_utils import run_bass_kernel_spmd

BATCH = 16384
MSG = 1000
NPAR = 256
NCORES = 8
ROWS = BATCH // NCORES  # 2048
P = 128
KB = 4  # k pair-blocks of 256; padded K = 1024
KPAD = KB * 2 * P
SC = 2  # m-chunks per pipeline unit

# test.py pokes these for profiling
TRACE = False
LAST_RESULT = None

_CACHE = {}

F8 = mybir.dt.float8e4
U16 = mybir.dt.uint16


def build_nc(rows=ROWS):
    """Emit the Bass/Tile IR for one core handling `rows` rows."""
    mch = rows // P
    n_super = mch // SC
    nc = bacc.Bacc("TRN2", target_bir_lowering=False, debug=False)
    msg = nc.dram_tensor("msg", [rows, MSG], mybir.dt.float32, kind="ExternalInput")
    gp = nc.dram_tensor("gp", [P, KB, 2, NPAR], F8, kind="ExternalInput")
    ident = nc.dram_tensor("ident", [P, P], F8, kind="ExternalInput")
    out = nc.dram_tensor(
        "out", [rows, MSG + NPAR], mybir.dt.float32, kind="ExternalOutput"
    )

    msg3 = msg[:, :].rearrange("(s c p) k -> s c p k", c=SC, p=P)
    out3 = out[:, :].rearrange("(s c p) k -> s c p k", c=SC, p=P)

    ch = SC // 2  # cast split point between ACT and DVE

    with tile.TileContext(nc) as tc:
        with (
            tc.tile_pool(name="gpool", bufs=1) as gpool,
            # every supertile's output-row tile is resident at once: all
            # loads dispatch upfront on the sync ring
            tc.tile_pool(name="opool", bufs=n_super) as opool,
            tc.tile_pool(name="fpool", bufs=2) as fpool,
            tc.tile_pool(name="tpool", bufs=2) as tpool,
            tc.tile_pool(name="cpool", bufs=2) as cpool,
            tc.tile_pool(name="epool", bufs=2) as epool,
            tc.tile_pool(name="tppool", bufs=2, space="PSUM") as tppool,
            tc.tile_pool(name="ppool", bufs=2, space="PSUM") as ppool,
        ):
            # Gp resident in SBUF: gsb[q, b, j, n] = Gp_padded[256*b + 2*q + j, n]
            gsb = gpool.tile([P, KB, 2, NPAR], F8)
            nc.sync.dma_start(out=gsb[:, :, :, :], in_=gp[:, :, :, :])
            idsb = gpool.tile([P, P], F8)
            nc.sync.dma_start(out=idsb[:, :], in_=ident[:, :])

            # all loads upfront on the sync ring (nothing else rides it)
            otiles = []
            for si in range(n_super):
                o = opool.tile([P, SC, MSG + NPAR], mybir.dt.float32, tag="o")
                nc.sync.dma_start(
                    out=o[:, :, 0:MSG],
                    in_=msg3[si, :, :, :].rearrange("c p k -> p c k"),
                )
                otiles.append(o)

            f8s, tps, ts, accs = {}, {}, {}, {}

            def emit_prep(si):
                # fp8 copy for the PE (0/1 exact), cast split ACT/DVE; pad
                # columns zeroed so the padded k-range contributes nothing
                f8 = fpool.tile([P, SC, KPAD], F8, tag="f8")
                nc.vector.memset(f8[:, :, MSG:KPAD], 0)
                nc.scalar.copy(f8[:, 0:ch, 0:MSG], otiles[si][:, 0:ch, 0:MSG])
                nc.vector.tensor_copy(
                    f8[:, ch:SC, 0:MSG], otiles[si][:, ch:SC, 0:MSG]
                )
                f8s[si] = f8

            def emit_transpose(si):
                # PE transpose of plain fp8 blocks -> PSUM (fp8 0/1 bytes are
                # always normal numbers -- no FTZ hazard). The fp8 transpose
                # datapath writes one value per 16-bit lane, so the output AP
                # has element step 2: tp[q, s, 2m] = msg[m, k-block s]
                f8 = f8s[si]
                tp = tppool.tile([P, SC * 8, 2 * P], F8, tag="tp")
                for c in range(SC):
                    for blk in range(8):
                        nc.tensor.transpose(
                            tp[:, c * 8 + blk, :].rearrange(
                                "q (m two) -> q m two", two=2
                            )[:, :, 0],
                            f8[:, c, 128 * blk : 128 * (blk + 1)],
                            idsb[:, :],
                        )
                tps[si] = tp

            def emit_evict(si):
                # transposed blocks PSUM -> SBUF on ACT (gather even bytes)
                t = tpool.tile([P, SC * 8, P], F8, tag="t")
                nc.scalar.copy(
                    t[:, :, :],
                    tps.pop(si)[:, :, :].rearrange(
                        "q s (m two) -> q s m two", two=2
                    )[:, :, :, 0],
                )
                ts[si] = t

            def emit_matmul(si):
                # DoubleRow fp8 matmuls: two adjacent true-transposed blocks
                # form the [128, 2, 128] block-layout weights AP (the layout
                # the dual-fp8 LDW ISA accepts), contracting k = 256g + 128i + q
                # against the matching host-swizzled Gp rows
                t = ts[si]
                acc = ppool.tile([P, SC * NPAR], mybir.dt.float32, tag="acc")
                for c in range(SC):
                    for g in range(KB):
                        nc.tensor.matmul(
                            acc[:, c * NPAR : (c + 1) * NPAR],
                            t[:, c * 8 + 2 * g : c * 8 + 2 * g + 2, :],
                            gsb[:, g, :, :],
                            start=(g == 0),
                            stop=(g == KB - 1),
                            perf_mode=mybir.MatmulPerfMode.DoubleRow,
                        )
                accs[si] = acc

            def emit_parity_store(si):
                o = otiles[si]
                # exact-integer f32 -> i32 eviction, mod 2 == AND 1, parity
                # i32 -> f32 into the output-row tile: all DVE
                ci = cpool.tile([P, SC, NPAR], mybir.dt.int32, tag="ci")
                nc.vector.tensor_copy(
                    ci[:, :, :].rearrange("p c n -> p (c n)"), accs.pop(si)[:, :]
                )
                e = epool.tile([P, SC, NPAR], mybir.dt.int32, tag="e")
                nc.vector.tensor_scalar(
                    e[:, :, :], ci[:, :, :], 1, None, mybir.AluOpType.bitwise_and
                )
                nc.vector.tensor_copy(o[:, :, MSG : MSG + NPAR], e[:, :, :])
                # plain f32 store via SWDGE: its own engine stream and queue
                nc.gpsimd.dma_start(
                    out=out3[si, :, :, :].rearrange("c p k -> p c k"),
                    in_=o[:, :, :],
                )

            # software-pipelined emission: per engine stream, everything a
            # unit's store needs (evict -> mm -> parity) is emitted before the
            # NEXT unit's load-gated casts, so early stores never queue behind
            # later loads' waits
            emit_prep(0)
            emit_transpose(0)
            for si in range(n_super):
                emit_evict(si)
                emit_matmul(si)
                emit_parity_store(si)
                if si + 1 < n_super:
                    emit_prep(si + 1)
                    emit_transpose(si + 1)

    nc.compile()
    return nc


def prep_gp(Gp):
    """Pad Gp to 1024 rows and swizzle to [128, 4, 2, 256] fp8:
    gsw[q, g, i, n] = Gp_pad[256*g + 128*i + q, n]
    """
    gp = np.asarray(Gp, dtype=np.float32)
    gp_pad = np.zeros((KPAD, NPAR), dtype=np.float32)
    gp_pad[:MSG] = gp
    gsw = gp_pad.reshape(KB, 2, P, NPAR).transpose(2, 0, 1, 3)
    return np.ascontiguousarray(gsw).astype(ml_dtypes.float8_e4m3)


def kernel(message_bits, Gp):
    global LAST_RESULT
    msg = np.ascontiguousarray(np.asarray(message_bits, dtype=np.float32))
    assert msg.shape == (BATCH, MSG), msg.shape
    gsw = prep_gp(Gp)
    ident = np.eye(P, dtype=np.float32).astype(ml_dtypes.float8_e4m3)

    if "nc" not in _CACHE:
        _CACHE["nc"] = build_nc()
    nc = _CACHE["nc"]

    in_maps = [
        {"msg": msg[i * ROWS : (i + 1) * ROWS], "gp": gsw, "ident": ident}
        for i in range(NCORES)
    ]
    res = run_bass_kernel_spmd(
        nc, in_maps, core_ids=list(range(NCORES)), trace=TRACE
    )
    LAST_RESULT = res
    return np.concatenate([r["out"] for r in res.results], axis=0)


# revision 13
# speedup vs baseline: 1.6555x; 1.0091x over previous
"""BCH/RS systematic encoder kernel for Trainium2 (8 NeuronCores, data parallel).

Computes out = concat([msg, (msg @ Gp) mod 2], axis=-1) for
msg [16384, 1000] f32 of 0/1 bits and Gp [1000, 256] f32 of 0/1 bits.

Design v7 (per core, 2048 rows, 4 supertiles of 4x128):
  - HWDGE plain f32 loads (2 MB each) straight into the f32 output-row tiles
    (copy-through region); SWDGE plain f32 stores of finished rows. Reads and
    writes share the ~358 GB/s HBM link: (8.19+10.29)MB/358 ~= 52 us is the
    floor, and with NO other DMA traffic the two streams fully interleave.
    (Tile serializes any xbar-transpose DMA against ALL concurrent DMAs --
    v5/v6 traces showed loads/stores blocked on transpose completion
    semaphores -- so all transposing happens on the PE instead.)
  - msg is cast f32 -> fp8e4 (0/1 exact), split between ACT and DVE.
  - The fp8 tile viewed as u16 PAIRS is transposed on the otherwise-idle PE
    (nc.tensor.transpose against a host-loaded identity), 16 blocks per
    supertile into PSUM, then evicted to SBUF by ACT: transposed partition q
    carries k = 2q, 2q+1 interleaved along m. u16 (not bf16) so no FTZ/NaN
    canonicalization can touch the fp8 pair bits.
  - Plain fp8 matmuls: per 256-k pair-block, the even/odd fp8 slot of each
    u16 is a stride-2 [128,128] weights AP, contracted against host-swizzled
    Gp rows; f32 PSUM accumulation is exact.
  - DVE evicts parity PSUM f32 -> i32, ANDs with 1 (mod 2), copies i32 -> f32
    into the output-row tile.
"""

import os
import sys

import numpy as np

if os.path.isdir("/opt/trn_rl_repo") and "/opt/trn_rl_repo" not in sys.path:
    sys.path.insert(0, "/opt/trn_rl_repo")

import ml_dtypes

import concourse.bacc as bacc
import concourse.mybir as mybir
import concourse.tile as tile
from concourse.bass_utils import run_bass_kernel_spmd

BATCH = 16384
MSG = 1000
NPAR = 256
NCORES = 8
ROWS = BATCH // NCORES  # 2048
P = 128
KB = 4  # k pair-blocks of 256; padded K = 1024
KPAD = KB * 2 * P
SC = 1  # m-chunks per pipeline unit

# test.py pokes these for profiling
TRACE = False
LAST_RESULT = None

_CACHE = {}

F8 = mybir.dt.float8e4
U16 = mybir.dt.uint16


def build_nc(rows=ROWS):
    """Emit the Bass/Tile IR for one core handling `rows` rows."""
    mch = rows // P
    n_super = mch // SC
    nc = bacc.Bacc("TRN2", target_bir_lowering=False, debug=False)
    msg = nc.dram_tensor("msg", [rows, MSG], mybir.dt.float32, kind="ExternalInput")
    gp = nc.dram_tensor("gp", [P, KB, 2, NPAR], F8, kind="ExternalInput")
    ident = nc.dram_tensor("ident", [P, P], F8, kind="ExternalInput")
    out = nc.dram_tensor(
        "out", [rows, MSG + NPAR], mybir.dt.float32, kind="ExternalOutput"
    )

    msg3 = msg[:, :].rearrange("(s c p) k -> s c p k", c=SC, p=P)
    out3 = out[:, :].rearrange("(s c p) k -> s c p k", c=SC, p=P)

    ch = SC // 2  # cast split point between ACT and DVE

    with tile.TileContext(nc) as tc:
        with (
            tc.tile_pool(name="gpool", bufs=1) as gpool,
            # every supertile's output-row tile is resident at once: all
            # loads dispatch upfront on the sync ring
            tc.tile_pool(name="opool", bufs=n_super) as opool,
            tc.tile_pool(name="fpool", bufs=2) as fpool,
            tc.tile_pool(name="tpool", bufs=2) as tpool,
            tc.tile_pool(name="cpool", bufs=2) as cpool,
            tc.tile_pool(name="epool", bufs=2) as epool,
            tc.tile_pool(name="tppool", bufs=2, space="PSUM") as tppool,
            tc.tile_pool(name="ppool", bufs=2, space="PSUM") as ppool,
        ):
            # Gp resident in SBUF: gsb[q, b, j, n] = Gp_padded[256*b + 2*q + j, n]
            gsb = gpool.tile([P, KB, 2, NPAR], F8)
            nc.sync.dma_start(out=gsb[:, :, :, :], in_=gp[:, :, :, :])
            idsb = gpool.tile([P, P], F8)
            nc.sync.dma_start(out=idsb[:, :], in_=ident[:, :])

            # all loads upfront on the sync ring (nothing else rides it)
            otiles = []
            for si in range(n_super):
                o = opool.tile([P, SC, MSG + NPAR], mybir.dt.float32, tag="o")
                nc.sync.dma_start(
                    out=o[:, :, 0:MSG],
                    in_=msg3[si, :, :, :].rearrange("c p k -> p c k"),
                )
                otiles.append(o)

            f8s, tps, ts, accs = {}, {}, {}, {}

            def emit_prep(si):
                # fp8 copy for the PE (0/1 exact), cast split ACT/DVE; pad
                # columns zeroed so the padded k-range contributes nothing
                f8 = fpool.tile([P, SC, KPAD], F8, tag="f8")
                nc.vector.memset(f8[:, :, MSG:KPAD], 0)
                nc.scalar.copy(f8[:, :, 0:500], otiles[si][:, :, 0:500])
                nc.vector.tensor_copy(
                    f8[:, :, 500:MSG], otiles[si][:, :, 500:MSG]
                )
                f8s[si] = f8

            def emit_transpose(si):
                # PE transpose of plain fp8 blocks -> PSUM (fp8 0/1 bytes are
                # always normal numbers -- no FTZ hazard). The fp8 transpose
                # datapath writes one value per 16-bit lane, so the output AP
                # has element step 2: tp[q, s, 2m] = msg[m, k-block s]
                f8 = f8s[si]
                tp = tppool.tile([P, SC * 8, 2 * P], F8, tag="tp")
                for c in range(SC):
                    for blk in range(8):
                        nc.tensor.transpose(
                            tp[:, c * 8 + blk, :].rearrange(
                                "q (m two) -> q m two", two=2
                            )[:, :, 0],
                            f8[:, c, 128 * blk : 128 * (blk + 1)],
                            idsb[:, :],
                        )
                tps[si] = tp

            def emit_evict(si):
                # transposed blocks PSUM -> SBUF on ACT (gather even bytes)
                t = tpool.tile([P, SC * 8, P], F8, tag="t")
                nc.scalar.copy(
                    t[:, :, :],
                    tps.pop(si)[:, :, :].rearrange(
                        "q s (m two) -> q s m two", two=2
                    )[:, :, :, 0],
                )
                ts[si] = t

            def emit_matmul(si):
                # DoubleRow fp8 matmuls: two adjacent true-transposed blocks
                # form the [128, 2, 128] block-layout weights AP (the layout
                # the dual-fp8 LDW ISA accepts), contracting k = 256g + 128i + q
                # against the matching host-swizzled Gp rows
                t = ts[si]
                acc = ppool.tile([P, SC * NPAR], mybir.dt.float32, tag="acc")
                for c in range(SC):
                    for g in range(KB):
                        nc.tensor.matmul(
                            acc[:, c * NPAR : (c + 1) * NPAR],
                            t[:, c * 8 + 2 * g : c * 8 + 2 * g + 2, :],
                            gsb[:, g, :, :],
                            start=(g == 0),
                            stop=(g == KB - 1),
                            perf_mode=mybir.MatmulPerfMode.DoubleRow,
                        )
                accs[si] = acc

            def emit_parity_store(si):
                o = otiles[si]
                # exact-integer f32 -> i32 eviction, mod 2 == AND 1, parity
                # i32 -> f32 into the output-row tile: all DVE
                ci = cpool.tile([P, SC, NPAR], mybir.dt.int32, tag="ci")
                nc.vector.tensor_copy(
                    ci[:, :, :].rearrange("p c n -> p (c n)"), accs.pop(si)[:, :]
                )
                e = epool.tile([P, SC, NPAR], mybir.dt.int32, tag="e")
                nc.vector.tensor_scalar(
                    e[:, :, :], ci[:, :, :], 1, None, mybir.AluOpType.bitwise_and
                )
                nc.vector.tensor_copy(o[:, :, MSG : MSG + NPAR], e[:, :, :])
                # plain f32 store via SWDGE: its own engine stream and queue
                nc.gpsimd.dma_start(
                    out=out3[si, :, :, :].rearrange("c p k -> p c k"),
                    in_=o[:, :, :],
                )

            # software-pipelined emission: per engine stream, everything a
            # unit's store needs (evict -> mm -> parity) is emitted before the
            # NEXT unit's load-gated casts, so early stores never queue behind
            # later loads' waits
            emit_prep(0)
            emit_transpose(0)
            for si in range(n_super):
                emit_evict(si)
                emit_matmul(si)
                emit_parity_store(si)
                if si + 1 < n_super:
                    emit_prep(si + 1)
                    emit_transpose(si + 1)

    nc.compile()
    return nc


def prep_gp(Gp):
    """Pad Gp to 1024 rows and swizzle to [128, 4, 2, 256] fp8:
    gsw[q, g, i, n] = Gp_pad[256*g + 128*i + q, n]
    """
    gp = np.asarray(Gp, dtype=np.float32)
    gp_pad = np.zeros((KPAD, NPAR), dtype=np.float32)
    gp_pad[:MSG] = gp
    gsw = gp_pad.reshape(KB, 2, P, NPAR).transpose(2, 0, 1, 3)
    return np.ascontiguousarray(gsw).astype(ml_dtypes.float8_e4m3)


def kernel(message_bits, Gp):
    global LAST_RESULT
    msg = np.ascontiguousarray(np.asarray(message_bits, dtype=np.float32))
    assert msg.shape == (BATCH, MSG), msg.shape
    gsw = prep_gp(Gp)
    ident = np.eye(P, dtype=np.float32).astype(ml_dtypes.float8_e4m3)

    if "nc" not in _CACHE:
        _CACHE["nc"] = build_nc()
    nc = _CACHE["nc"]

    in_maps = [
        {"msg": msg[i * ROWS : (i + 1) * ROWS], "gp": gsw, "ident": ident}
        for i in range(NCORES)
    ]
    res = run_bass_kernel_spmd(
        nc, in_maps, core_ids=list(range(NCORES)), trace=TRACE
    )
    LAST_RESULT = res
    return np.concatenate([r["out"] for r in res.results], axis=0)
